# revision 1
# baseline (speedup 1.0000x reference)
"""DLRM dot-interaction kernel for Trainium2 (8 NeuronCores, batch-sharded).

Per sample b: T = concat(dense[b], embs[b]) -> [27, 128]; Z = T @ T^T;
output = strict upper triangle of Z -> [351] fp32.

Per-core plan (2048 samples, 16 blocks of 128):
  - SWDGE cast-DMA loads 2 blocks at a time as [128 b, (f,d)] fp16.
  - PE transposes each [128 b, 128 d] feature slab into PSUM; DVE/ACT copy
    into f-major Tt [128 d, f*128+b] fp16 (contiguous copies).
  - Per-sample fp16 matmul: lhsT = rhs = strided AP [128 d, 32 f] (27 real
    features + 5 zero pads); out -> PSUM [32, 32] at partition 32*(b%4)
    (col-group tiling, 4 samples per PSUM partition dim).
  - ACT copies Z PSUM -> SBUF Zs [(g,m) part, (blk,q,n)] fp32, half-core span.
  - Triu extraction: SWDGE bounces Zs to DRAM scratch (full rows, big
    descriptors); then per (m, half) one HWDGE DRAM->DRAM gather DMA with
    1024 descriptors (spreads over ~8-16 DMA engines) packs z[m, m+1:27]
    runs into out[b, off_m:...].
"""

import numpy as np

B, NUM_EMBS, D = 16384, 26, 128
N_CORES = 8
BC = B // N_CORES  # 2048 samples per core
BLK = 128          # samples per block
NF = NUM_EMBS + 1  # 27 features
FP = 32            # feature pitch (27 + 5 pad)
NPAIR = NF * (NF - 1) // 2  # 351

_CACHE = {}


def build(bc=BC):
    import concourse.bacc as bacc
    import concourse.mybir as mybir
    from concourse.tile import TileContext
    from concourse.masks import make_identity

    fp16 = mybir.dt.float16
    fp32 = mybir.dt.float32

    nc = bacc.Bacc("TRN2", target_bir_lowering=False, debug=False)
    dense_t = nc.dram_tensor("dense", (bc, D), fp32, kind="ExternalInput")
    embs_t = nc.dram_tensor("embs", (bc, NUM_EMBS, D), fp32, kind="ExternalInput")
    out_t = nc.dram_tensor("out", (bc, NPAIR), fp32, kind="ExternalOutput")

    nblk = bc // BLK
    assert nblk % 2 == 0
    QBLK = min(4, nblk)  # blocks per quarter-group (Zs/scratch granularity)
    QG = 16              # 4-sample groups per PSUM Z tile

    # Input load plan: small groups first (fast pipeline start), 4-block
    # groups at steady state (fewer SWDGE generations).
    groups = []
    b = 0
    head = [1, 1, 2]
    while b < nblk:
        sz = min(head.pop(0) if head else 4, nblk - b)
        groups.append((b, sz))
        b += sz
    g_of = {}
    for gs, sz in groups:
        for i in range(sz):
            g_of[gs + i] = (gs, sz)

    with TileContext(nc) as tc:
        with (
            tc.tile_pool(name="consts", bufs=1) as consts,
            tc.tile_pool(name="xin", bufs=2) as xpool,
            tc.tile_pool(name="tt", bufs=4) as ttpool,
            tc.tile_pool(name="zsb", bufs=6) as zpool,
            tc.tile_pool(name="zb", bufs=3) as zbpool,
            tc.tile_pool(name="pk", bufs=3) as pkpool,
            tc.tile_pool(name="tp", bufs=4, space="PSUM") as tppool,
            tc.tile_pool(name="zp", bufs=4, space="PSUM") as zppool,
            tc.tile_pool(name="dscr", bufs=8, space="DRAM") as dpool,
        ):
            ident = consts.tile([128, 128], fp16)
            make_identity(nc, ident)

            dview = dense_t.ap()  # [bc, 128]
            eview = embs_t.ap().rearrange("b f d -> b (f d)")  # [bc, 3328]
            oview = out_t.ap()  # [bc, 351]

            X = None
            for qtr in range(nblk // QBLK):
                scr_ts = []  # per-block scratch tiles for this quarter
                for pairi in range(max(1, QBLK // 2)):
                    npair = min(2, QBLK)
                    tts = []
                    # ---- phase 1: load + transpose for the block pair ----
                    for sub in range(npair):
                        blki = pairi * 2 + sub
                        blk = qtr * QBLK + blki
                        b0 = blk * BLK
                        gs, gsz = g_of[blk]
                        if blk == gs:
                            # SWDGE load casts fp32 -> fp16 at full rate
                            X = xpool.tile([BLK, gsz * NF * D], fp16, tag="X")
                            dsrc = dview[gs * BLK : (gs + gsz) * BLK].rearrange(
                                "(t b) d -> b t d", t=gsz
                            )  # [128, gsz, 128]
                            xd = X.rearrange("b (t c) -> b t c", t=gsz)
                            nc.gpsimd.dma_start(out=xd[:, :, 0:D], in_=dsrc)
                            esrc = eview[gs * BLK : (gs + gsz) * BLK].rearrange(
                                "(t b) c -> b t c", t=gsz
                            )  # [128, gsz, 3328]
                            nc.gpsimd.dma_start(out=xd[:, :, D:], in_=esrc)
                        xoff = (blk - gs) * NF * D

                        Tt = ttpool.tile([128, FP * D], fp16, tag="Tt")
                        # zero pad features f=27..31 (cols 3456:4096)
                        nc.gpsimd.memset(Tt[:, NF * D :], 0.0)
                        nchunk = (NF + 7) // 8
                        for ci in range(nchunk):
                            c0 = ci * 8
                            cf = min(8, NF - c0)
                            tp = tppool.tile([128, 8 * BLK], fp16, tag="tp")
                            for j in range(cf):
                                f = c0 + j
                                nc.tensor.transpose(
                                    tp[:, j * BLK : (j + 1) * BLK],
                                    X[:, xoff + f * D : xoff + (f + 1) * D],
                                    ident,
                                )
                            dst = Tt[:, c0 * BLK : (c0 + cf) * BLK]
                            src = tp[:, : cf * BLK]
                            if ci % 4 < 3:
                                nc.vector.tensor_copy(out=dst, in_=src)
                            else:
                                nc.scalar.copy(dst, src)
                        tts.append((blki, Tt))

                    # ---- phase 2: per-sample Gram matmuls (dense PE burst) --
                    for blki, Tt in tts:
                        Ttr = Tt.rearrange("d (f b) -> d b f", b=BLK)
                        nq = BLK // 4  # 32 groups of 4 samples
                        # First pair writes full 32-row strips so the PSUM
                        # pad partitions (27-31) are initialized once; after
                        # that, 27-column weights skip 5 LDWEIGHTS columns
                        # per sample and pads keep stale (unread) data.
                        mw = FP
                        Zs_t = zpool.tile([128, nq * FP], fp16, tag="Zs")
                        for qt in range(0, nq, QG):
                            zp = zppool.tile([128, QG * FP], fp32, tag="zp")
                            for q in range(QG):
                                for g in range(4):
                                    bloc = (qt + q) * 4 + g
                                    wop = Ttr[:, bloc, :mw]  # [128 d, mw f]
                                    mop = Ttr[:, bloc, :]    # [128 d, 32 f]
                                    nc.tensor.matmul(
                                        zp[
                                            32 * g : 32 * g + mw,
                                            q * FP : (q + 1) * FP,
                                        ],
                                        wop,
                                        mop,
                                        start=True,
                                        stop=True,
                                        tile_position=(0, 32 * g),
                                    )
                            # copy Z PSUM -> SBUF block buffer, cast fp16
                            zcol0 = qt * FP
                            zdst = Zs_t[:, zcol0 : zcol0 + QG * FP]
                            if qt == 0:
                                nc.scalar.copy(zdst, zp[:, : QG * FP])
                            else:
                                nc.vector.tensor_copy(
                                    out=zdst, in_=zp[:, : QG * FP]
                                )
                        # ---- bounce this block's Z to DRAM scratch in raw
                        # partition order (one full-partition DMA, 2KB runs;
                        # per-block granularity keeps the chain pipelined)
                        scr_t = dpool.tile([128, nq * FP], fp16, tag="scr")
                        nc.gpsimd.dma_start(out=scr_t[:, :], in_=Zs_t[:, :])
                        scr_ts.append(scr_t)

                # ---- reload as [(g, qlo) part, (t, m, n)] ----
                # sample s = q*4+g with q = t*32+qlo -> partition g*32+qlo,
                # column block t. 128B runs, wide HWDGE fanout.
                Zb = zbpool.tile([128, QBLK * NF * FP], fp16, tag="Zb")
                zb5 = Zb.rearrange(
                    "(g qlo) (t m n) -> g qlo t m n", g=4, t=QBLK, n=FP
                )  # [4, 32, t, 27, 32]
                for t, scr_t in enumerate(scr_ts):
                    sct = scr_t.rearrange(
                        "(g m) (q n) -> g q m n", g=4, n=FP
                    )  # [4, 32q, 32m, 32n]
                    for g in range(4):
                        nc.sync.dma_start(out=zb5[g, :, t], in_=sct[g, :, :NF, :])

                # ---- pack triu (QBLK tiles wide per copy, DVE) ----
                Pk = pkpool.tile([128, QBLK * NPAIR], fp32, tag="Pk")
                zbp = Zb.rearrange(
                    "p (t c) -> p t c", t=QBLK
                )  # [128, t, 864+pad]
                pkp = Pk.rearrange("p (t c) -> p t c", t=QBLK)  # [128, t, 351]
                off = 0
                for m in range(NF - 1):
                    ln = NF - 1 - m
                    src = zbp[:, :, m * FP + m + 1 : m * FP + NF]
                    dst = pkp[:, :, off : off + ln]
                    # fp16 -> fp32 cast happens in the copy
                    if m % 3 == 2:
                        nc.scalar.copy(dst, src)
                    else:
                        nc.vector.tensor_copy(out=dst, in_=src)
                    off += ln

                # ---- output: per-g HWDGE DMA, 1404B runs ----
                b0q = qtr * QBLK * BLK
                ovq = oview[b0q : b0q + QBLK * BLK].rearrange(
                    "(t qlo g) p -> g qlo t p", g=4, t=QBLK
                )  # [4, 32, t, 351]
                pk4 = pkp.rearrange("(g qlo) t c -> g qlo t c", g=4)
                for g in range(4):
                    eng = nc.sync if g % 2 == 0 else nc.scalar
                    eng.dma_start(out=ovq[g], in_=pk4[g])

    nc.compile()
    return nc


def _get(bc=BC):
    if bc not in _CACHE:
        _CACHE[bc] = build(bc)
    return _CACHE[bc]


def kernel(dense: np.ndarray, embs: np.ndarray) -> np.ndarray:
    from concourse import bass_utils

    dense = np.ascontiguousarray(np.asarray(dense, dtype=np.float32))
    embs = np.ascontiguousarray(np.asarray(embs, dtype=np.float32))
    assert dense.shape == (B, D) and embs.shape == (B, NUM_EMBS, D)

    nc = _get()
    dsh = dense.reshape(N_CORES, BC, D)
    esh = embs.reshape(N_CORES, BC, NUM_EMBS, D)
    in_maps = [{"dense": dsh[i], "embs": esh[i]} for i in range(N_CORES)]
    res = bass_utils.run_bass_kernel_spmd(nc, in_maps, core_ids=list(range(N_CORES)))
    return np.concatenate([r["out"] for r in res.results], axis=0)



# revision 8
# speedup vs baseline: 118.6888x; 118.6888x over previous
"""DLRM dot-interaction kernel for Trainium2 (8 NeuronCores, batch-sharded).

Per sample b: T = concat(dense[b], embs[b]) -> [27, 128]; Z = T @ T^T;
output = strict upper triangle of Z -> [351] fp32.

Per-core plan (2048 samples, 16 blocks of 128):
  - SWDGE cast-DMA loads block groups as X [128 b, (t,f,d)] fp16.
  - PE transposes each [128 b, 128 d] feature slab into fp16 PSUM; DVE/ACT
    copy into f-major Tt [128 d, f*128+b] fp16 (contiguous copies).
  - Per-sample fp16 Gram matmul: lhsT = Ttr[:, b, :27] (27-col weight),
    moving = Ttr[:, b, :32]; out -> PSUM [27, 32] at col-tile 32*(b%4).
  - ACT/DVE copy Z PSUM -> SBUF Zs [(g,m) part, (q,n)] fp16 per block.
  - On-chip shuffle: SBUF->SBUF HWDGE DMA with partition-crossing views
    rearranges Zs into sample-major Zb [(g,q) part, (t,m,n)] fp16 (no DRAM
    bounce).
  - Triu pack: 26 DVE/ACT copies (fp16->fp32 cast) -> Pk [b, 4*351];
    one 719KB HWDGE DMA per 4-block quarter writes out[b, 351].
"""

import numpy as np

B, NUM_EMBS, D = 16384, 26, 128
N_CORES = 8
BC = B // N_CORES  # 2048 samples per core
BLK = 128          # samples per block
NF = NUM_EMBS + 1  # 27 features
FP = 32            # feature pitch (27 + 5 pad)
NPAIR = NF * (NF - 1) // 2  # 351

_CACHE = {}


def build(bc=BC):
    import concourse.bacc as bacc
    import concourse.mybir as mybir
    from concourse.tile import TileContext
    from concourse.masks import make_identity

    fp16 = mybir.dt.float16
    fp32 = mybir.dt.float32

    nc = bacc.Bacc("TRN2", target_bir_lowering=False, debug=False)
    dense_t = nc.dram_tensor("dense", (bc, D), fp32, kind="ExternalInput")
    embs_t = nc.dram_tensor("embs", (bc, NUM_EMBS, D), fp32, kind="ExternalInput")
    out_t = nc.dram_tensor("out", (bc, NPAIR), fp32, kind="ExternalOutput")

    nblk = bc // BLK
    QBLK = 4  # blocks per quarter (pack/out granularity)

    # Input load plan: small groups first (fast pipeline start), then 4-block
    # groups (big SWDGE transfers).
    groups = []
    b = 0
    head = [1, 1, 2]
    while b < nblk:
        sz = min(head.pop(0) if head else 4, nblk - b)
        groups.append((b, sz))
        b += sz
    g_of = {}
    for gs, sz in groups:
        for i in range(sz):
            g_of[gs + i] = (gs, sz)

    with TileContext(nc) as tc:
        with (
            tc.tile_pool(name="consts", bufs=1) as consts,
            tc.tile_pool(name="xin", bufs=2) as xpool,
            tc.tile_pool(name="tt", bufs=3) as ttpool,
            tc.tile_pool(name="zsb", bufs=3) as zpool,
            tc.tile_pool(name="zb", bufs=2) as zbpool,
            tc.tile_pool(name="pk", bufs=2) as pkpool,
            tc.tile_pool(name="tp", bufs=3, space="PSUM") as tppool,
            tc.tile_pool(name="zp", bufs=4, space="PSUM") as zppool,
            tc.tile_pool(name="dscr", bufs=6, space="DRAM") as dpool,
        ):
            ident = consts.tile([128, 128], fp16)
            make_identity(nc, ident)

            dview = dense_t.ap()  # [bc, 128]
            eview = embs_t.ap().rearrange("b f d -> b (f d)")  # [bc, 3328]
            oview = out_t.ap()  # [bc, 351]

            X = None
            cp_i = 0  # round-robin counter for DVE/ACT copies

            def copy(out, in_):
                nonlocal cp_i
                cp_i += 1
                if cp_i % 3 < 2:
                    nc.vector.tensor_copy(out=out, in_=in_)
                else:
                    nc.scalar.copy(out, in_)

            for qtr in range(nblk // QBLK):
                Zb = zbpool.tile([128, QBLK * NF * FP], fp16, tag="Zb")
                zb5 = Zb.rearrange(
                    "(g q) (t m n) -> g q t m n", g=4, t=QBLK, n=FP
                )  # [4, 32, t, 27, 32]

                for t in range(QBLK):
                    blk = qtr * QBLK + t
                    gs, gsz = g_of[blk]
                    if blk == gs:
                        # SWDGE load casts fp32 -> fp16 at full rate
                        X = xpool.tile([BLK, gsz * NF * D], fp16, tag="X")
                        dsrc = dview[gs * BLK : (gs + gsz) * BLK].rearrange(
                            "(t b) d -> b t d", t=gsz
                        )  # [128, gsz, 128]
                        xd = X.rearrange("b (t c) -> b t c", t=gsz)
                        nc.gpsimd.dma_start(out=xd[:, :, 0:D], in_=dsrc)
                        esrc = eview[gs * BLK : (gs + gsz) * BLK].rearrange(
                            "(t b) c -> b t c", t=gsz
                        )  # [128, gsz, 3328]
                        nc.gpsimd.dma_start(out=xd[:, :, D:], in_=esrc)
                    xoff = (blk - gs) * NF * D

                    # ---- transposes: X slabs -> fp16 PSUM -> f-major Tt ----
                    Tt = ttpool.tile([128, FP * D], fp16, tag="Tt")
                    # zero pad features f=27..31 (cols 3456:4095): moving pads
                    nc.gpsimd.memset(Tt[:, NF * D :], 0.0)
                    for ci, (c0, cf) in enumerate(((0, 8), (8, 8), (16, 8), (24, 3))):
                        tp = tppool.tile([128, 8 * BLK], fp16, tag="tp")
                        for j in range(cf):
                            f = c0 + j
                            nc.tensor.transpose(
                                tp[:, j * BLK : (j + 1) * BLK],
                                X[:, xoff + f * D : xoff + (f + 1) * D],
                                ident,
                            )
                        copy(Tt[:, c0 * BLK : (c0 + cf) * BLK], tp[:, : cf * BLK])

                    # ---- per-sample Gram matmuls (dense PE burst) ----
                    Ttr = Tt.rearrange("d (f b) -> d b f", b=BLK)  # [d, b, 32f]
                    Zs = zpool.tile([128, 32 * FP], fp16, tag="Zs")
                    for qt in range(0, 32, 16):
                        zp = zppool.tile([128, 16 * FP], fp32, tag="zp")
                        for q in range(16):
                            for g in range(4):
                                bloc = (qt + q) * 4 + g
                                nc.tensor.matmul(
                                    zp[32 * g : 32 * g + NF, q * FP : (q + 1) * FP],
                                    Ttr[:, bloc, :NF],   # 27-col weight
                                    Ttr[:, bloc, :],     # 32-col moving
                                    start=True,
                                    stop=True,
                                    tile_position=(0, 32 * g),
                                )
                        copy(Zs[:, qt * FP : (qt + 16) * FP], zp[:, : 16 * FP])

                    # ---- bounce Zs to DRAM scratch (2KB runs/partition),
                    # then gather back sample-major (64B runs) per block ----
                    scr = dpool.tile([128, 32 * FP], fp16, tag="scr")
                    nc.gpsimd.dma_start(out=scr[:, :], in_=Zs[:, :])
                    sct = scr.rearrange(
                        "(g m) (q n) -> g q m n", g=4, n=FP
                    )  # [4, 32q, 32m, 32n] in DRAM
                    for g in range(4):
                        eng = nc.sync if (t + g) % 2 == 0 else nc.scalar
                        eng.dma_start(out=zb5[g, :, t], in_=sct[g, :, :NF, :])

                # ---- pack triu (QBLK tiles wide per copy) ----
                Pk = pkpool.tile([128, QBLK * NPAIR], fp32, tag="Pk")
                zbp = Zb.rearrange("p (t c) -> p t c", t=QBLK)  # [128, t, 864]
                pkp = Pk.rearrange("p (t c) -> p t c", t=QBLK)  # [128, t, 351]
                off = 0
                for m in range(NF - 1):
                    ln = NF - 1 - m
                    src = zbp[:, :, m * FP + m + 1 : m * FP + NF]
                    dst = pkp[:, :, off : off + ln]
                    # fp16 -> fp32 cast happens in the copy
                    copy(dst, src)
                    off += ln

                # ---- output: one 719KB HWDGE DMA per quarter ----
                b0q = qtr * QBLK * BLK
                ovq = oview[b0q : b0q + QBLK * BLK].rearrange(
                    "(t q g) p -> g q t p", g=4, t=QBLK
                )  # [4, 32, t, 351]
                pk4 = Pk.rearrange("(g q) (t c) -> g q t c", g=4, t=QBLK)
                for g in range(4):
                    eng = nc.sync if g % 2 == 0 else nc.scalar
                    eng.dma_start(out=ovq[g], in_=pk4[g])

    nc.compile()
    return nc


def _get(bc=BC):
    if bc not in _CACHE:
        _CACHE[bc] = build(bc)
    return _CACHE[bc]


def kernel(dense: np.ndarray, embs: np.ndarray) -> np.ndarray:
    from concourse import bass_utils

    dense = np.ascontiguousarray(np.asarray(dense, dtype=np.float32))
    embs = np.ascontiguousarray(np.asarray(embs, dtype=np.float32))
    assert dense.shape == (B, D) and embs.shape == (B, NUM_EMBS, D)

    nc = _get()
    dsh = dense.reshape(N_CORES, BC, D)
    esh = embs.reshape(N_CORES, BC, NUM_EMBS, D)
    in_maps = [{"dense": dsh[i], "embs": esh[i]} for i in range(N_CORES)]
    res = bass_utils.run_bass_kernel_spmd(nc, in_maps, core_ids=list(range(N_CORES)))
    return np.concatenate([r["out"] for r in res.results], axis=0)


# revision 10
# speedup vs baseline: 125.6303x; 1.0585x over previous
"""DLRM dot-interaction kernel for Trainium2 (8 NeuronCores, batch-sharded).

Per sample b: T = concat(dense[b], embs[b]) -> [27, 128]; Z = T @ T^T;
output = strict upper triangle of Z -> [351] fp32.

Per-core plan (2048 samples, 16 blocks of 128):
  - SWDGE cast-DMA loads block groups as X [128 b, (t,f,d)] fp16 (gpsimd).
  - PE transposes each [128 b, 128 d] feature slab into fp16 PSUM; DVE
    copies into f-major Tt [128 d, f*128+b] fp16 (contiguous, 2x mode).
  - Per-sample fp16 Gram matmul: lhsT = Ttr[:, b, :27] (27-col weight),
    moving = Ttr[:, b, :32]; out -> PSUM [27, 32] at col-tile 32*(b%4);
    DVE copies Z PSUM -> SBUF Zs [(g,m) part, (q,n)] fp16 per block.
  - Engine separation: DVE owns only the PE-critical copies (Tt, Zs);
    ACT owns only the triu-pack copies; sync owns scratch write+gather
    (FIFO-ordered); gpsimd owns input loads + output cast DMAs.
  - Bounce: one HWDGE write per block to m-major DRAM scratch
    (addr = m*4096 + g*1024 + q*32 + n, 2KB runs), one HWDGE gather back
    as sample-major Zb [(g,q) part, (t,m,n)] fp16 (64B runs, fused
    partition stride).
  - Triu pack: 26 ACT copies per 4-block quarter -> Pk16 [b, 4*351] fp16;
    4 SWDGE cast-DMAs (fp16->fp32) per quarter write out[b, 351].
"""

import numpy as np

B, NUM_EMBS, D = 16384, 26, 128
N_CORES = 8
BC = B // N_CORES  # 2048 samples per core
BLK = 128          # samples per block
NF = NUM_EMBS + 1  # 27 features
FP = 32            # feature pitch (27 + 5 pad)
NPAIR = NF * (NF - 1) // 2  # 351

_CACHE = {}


def build(bc=BC):
    import concourse.bacc as bacc
    import concourse.mybir as mybir
    from concourse.tile import TileContext
    from concourse.masks import make_identity

    fp16 = mybir.dt.float16
    fp32 = mybir.dt.float32

    nc = bacc.Bacc("TRN2", target_bir_lowering=False, debug=False)
    dense_t = nc.dram_tensor("dense", (bc, D), fp32, kind="ExternalInput")
    embs_t = nc.dram_tensor("embs", (bc, NUM_EMBS, D), fp32, kind="ExternalInput")
    out_t = nc.dram_tensor("out", (bc, NPAIR), fp32, kind="ExternalOutput")

    nblk = bc // BLK
    QBLK = 4  # blocks per quarter (pack/out granularity)

    # Input load plan: small groups first (fast pipeline start), then 4-block
    # groups (big SWDGE transfers).
    groups = []
    b = 0
    head = [1, 1, 2]
    while b < nblk:
        sz = min(head.pop(0) if head else 4, nblk - b)
        groups.append((b, sz))
        b += sz
    g_of = {}
    for gs, sz in groups:
        for i in range(sz):
            g_of[gs + i] = (gs, sz)

    with TileContext(nc) as tc:
        with (
            tc.tile_pool(name="consts", bufs=1) as consts,
            tc.tile_pool(name="xin", bufs=3) as xpool,
            tc.tile_pool(name="tt", bufs=3) as ttpool,
            tc.tile_pool(name="zsb", bufs=6) as zpool,
            tc.tile_pool(name="zb", bufs=2) as zbpool,
            tc.tile_pool(name="pk", bufs=2) as pkpool,
            tc.tile_pool(name="tp", bufs=4, space="PSUM") as tppool,
            tc.tile_pool(name="zp", bufs=4, space="PSUM") as zppool,
            tc.tile_pool(name="dscr", bufs=6, space="DRAM") as dpool,
        ):
            ident = consts.tile([128, 128], fp16)
            make_identity(nc, ident)

            dview = dense_t.ap()  # [bc, 128]
            eview = embs_t.ap().rearrange("b f d -> b (f d)")  # [bc, 3328]
            oview = out_t.ap()  # [bc, 351]

            X = None
            for qtr in range(nblk // QBLK):
                Zb = zbpool.tile([128, QBLK * NF * FP], fp16, tag="Zb")

                for t in range(QBLK):
                    blk = qtr * QBLK + t
                    gs, gsz = g_of[blk]
                    if blk == gs:
                        # SWDGE load casts fp32 -> fp16 at full rate
                        X = xpool.tile([BLK, gsz * NF * D], fp16, tag="X")
                        dsrc = dview[gs * BLK : (gs + gsz) * BLK].rearrange(
                            "(t b) d -> b t d", t=gsz
                        )  # [128, gsz, 128]
                        xd = X.rearrange("b (t c) -> b t c", t=gsz)
                        nc.gpsimd.dma_start(out=xd[:, :, 0:D], in_=dsrc)
                        esrc = eview[gs * BLK : (gs + gsz) * BLK].rearrange(
                            "(t b) c -> b t c", t=gsz
                        )  # [128, gsz, 3328]
                        nc.gpsimd.dma_start(out=xd[:, :, D:], in_=esrc)
                    xoff = (blk - gs) * NF * D

                    # ---- transposes: X slabs -> fp16 PSUM -> f-major Tt ----
                    # (pad cols f=27..31 stay garbage; they only ever reach
                    # PSUM cols n>=27 / Zs rows m>=27, which are never read)
                    Tt = ttpool.tile([128, FP * D], fp16, tag="Tt")
                    for c0, cf in ((0, 8), (8, 8), (16, 8), (24, 3)):
                        tp = tppool.tile([128, 8 * BLK], fp16, tag="tp")
                        for j in range(cf):
                            f = c0 + j
                            nc.tensor.transpose(
                                tp[:, j * BLK : (j + 1) * BLK],
                                X[:, xoff + f * D : xoff + (f + 1) * D],
                                ident,
                            )
                        nc.vector.tensor_copy(
                            out=Tt[:, c0 * BLK : (c0 + cf) * BLK],
                            in_=tp[:, : cf * BLK],
                        )

                    # ---- per-sample Gram matmuls (dense PE burst) ----
                    Ttr = Tt.rearrange("d (f b) -> d b f", b=BLK)  # [d, b, 32f]
                    Zs = zpool.tile([128, 32 * FP], fp16, tag="Zs")
                    for qt in range(0, 32, 16):
                        zp = zppool.tile([128, 16 * FP], fp32, tag="zp")
                        for q in range(16):
                            for g in range(4):
                                bloc = (qt + q) * 4 + g
                                nc.tensor.matmul(
                                    zp[32 * g : 32 * g + NF, q * FP : (q + 1) * FP],
                                    Ttr[:, bloc, :NF],   # 27-col weight
                                    Ttr[:, bloc, :],     # 32-col moving
                                    start=True,
                                    stop=True,
                                    tile_position=(0, 32 * g),
                                )
                        nc.vector.tensor_copy(
                            out=Zs[:, qt * FP : (qt + 16) * FP],
                            in_=zp[:, : 16 * FP],
                        )

                    # ---- bounce: Zs -> m-major DRAM scratch (2KB runs),
                    # gather back sample-major into Zb[:, t] (64B runs) ----
                    scr = dpool.tile([128, 32 * FP], fp16, tag="scr")
                    # write: row (m*4+g) <- Zs partition (g*32+m)
                    wdst = scr.rearrange("(m g) c -> g m c", g=4)
                    nc.sync.dma_start(out=wdst, in_=Zs[:, :])
                    # read: [p'=(a b)=g*32+q, m, n] with addr m*4096+p'*32+n
                    rsrc = scr.rearrange("(m a) (b n) -> (a b) m n", a=4, n=FP)
                    zbt = Zb[:, t * NF * FP : (t + 1) * NF * FP].rearrange(
                        "p (m n) -> p m n", n=FP
                    )
                    nc.sync.dma_start(out=zbt, in_=rsrc[:, :NF, :])

                # ---- pack triu (ACT only; fp16 -> fp16) ----
                Pk = pkpool.tile([128, QBLK * NPAIR], fp16, tag="Pk")
                zbp = Zb.rearrange("p (t c) -> p t c", t=QBLK)  # [128, t, 864]
                pkp = Pk.rearrange("p (t c) -> p t c", t=QBLK)  # [128, t, 351]
                off = 0
                for m in range(NF - 1):
                    ln = NF - 1 - m
                    nc.scalar.copy(
                        pkp[:, :, off : off + ln],
                        zbp[:, :, m * FP + m + 1 : m * FP + NF],
                    )
                    off += ln

                # ---- output: per-g SWDGE cast DMA (fp16 -> fp32) ----
                b0q = qtr * QBLK * BLK
                ovq = oview[b0q : b0q + QBLK * BLK].rearrange(
                    "(t q g) p -> g q t p", g=4, t=QBLK
                )  # [4, 32, t, 351]
                pk4 = Pk.rearrange("(g q) (t c) -> g q t c", g=4, t=QBLK)
                for g in range(4):
                    nc.gpsimd.dma_start(out=ovq[g], in_=pk4[g])

    nc.compile()
    return nc


def _get(bc=BC):
    if bc not in _CACHE:
        _CACHE[bc] = build(bc)
    return _CACHE[bc]


def kernel(dense: np.ndarray, embs: np.ndarray) -> np.ndarray:
    from concourse import bass_utils

    dense = np.ascontiguousarray(np.asarray(dense, dtype=np.float32))
    embs = np.ascontiguousarray(np.asarray(embs, dtype=np.float32))
    assert dense.shape == (B, D) and embs.shape == (B, NUM_EMBS, D)

    nc = _get()
    dsh = dense.reshape(N_CORES, BC, D)
    esh = embs.reshape(N_CORES, BC, NUM_EMBS, D)
    in_maps = [{"dense": dsh[i], "embs": esh[i]} for i in range(N_CORES)]
    res = bass_utils.run_bass_kernel_spmd(nc, in_maps, core_ids=list(range(N_CORES)))
    return np.concatenate([r["out"] for r in res.results], axis=0)
